# revision 29
# baseline (speedup 1.0000x reference)
"""Multi-head attention (raw-reshape variant) on 8 trn2 NeuronCores.

Shapes: B=2, S=2048, D=1024, H=16, dh=64.  The reference uses a raw
reshape (B,S,D)->(B,H,S,dh), so head h only sees projected rows
[128h, 128h+128).  Core c handles b=c//4 and the 4 heads of seq-block
c%4.  No collectives.

All data tensors are fp16: softmax-weight quantization noise passes
through to the output at full strength (weights multiply V and both
signal and output shrink by the same averaging factor), so fp8 anywhere
on the value path costs ~3.6% output error -- over the 2e-2 gate.

v3 structure (vs the v1 baseline):
  1. Q/K proj with M=128 psum tiles [dm-pair-block, 512 rows] (half the
     PE passes of the M=64 parity-packed v1 layout); the fold to
     Qt/Kt[64hp+d, 2048g+128t+r] runs as 64 partition-moving
     psum->sbuf casts split between DVE and ACT.
  2. V proj natural [rows, dm] -> vaug fp16 with ones columns (the PV
     matmul emits the softmax denominator for free).
  3. Per (pair, q-half, t): St = Kt_t.T @ Qt (K=64); exp on ACT with
     the e^-4 softmax shift baked into the mask values; mask-mul on
     DVE (fp16 2x mode) with a few steps per phase on GPSIMD; PV
     accumulates psO (K=128).
  4. Normalize via DVE reciprocal + mul, then repack the stack to a
     two-t-block [128 = (tpar, d)] layout so the output projection
     contracts K=128 per pass; fused 1/256 rescale on the psum->sbuf
     copy; one wo load for the whole kernel.
"""

import numpy as np

import concourse.bass as bass
import concourse.mybir as mybir
import concourse.tile as tile
from concourse import bacc
from concourse.bass_utils import run_bass_kernel_spmd

F32 = mybir.dt.float32
F16 = mybir.dt.float16

B, S, D, H, DH = 2, 2048, 1024, 16, 64
N_CORES = 8
CORE_ROWS = 512
N_PAIRS = 4
WSCALE = 16.0
EXP_SCALE = 0.125 / (WSCALE * WSCALE)
MASK_SHIFT = float(np.exp(-4.0))
GP_STEPS = ()                    # gpsimd mask-mul offload (off: stalls PV)

_NC = None


def _build_program():
    nc = bacc.Bacc()

    # per contraction chunk k: [w chunk (1024 dm) | x chunk (512 rows)]
    qasm = nc.dram_tensor("qasm", [8, 128, 1536], F16, kind="ExternalInput")
    kasm = nc.dram_tensor("kasm", [8, 128, 1536], F16, kind="ExternalInput")
    vasm = nc.dram_tensor("vasm", [8, 128, 1536], F16, kind="ExternalInput")
    wodr = nc.dram_tensor("wodr", [128, 8192], F16, kind="ExternalInput")
    maskc_d = nc.dram_tensor("maskc", [S, S], mybir.dt.uint8, kind="ExternalInput")
    out_d = nc.dram_tensor("out", [CORE_ROWS, D], F32, kind="ExternalOutput")

    with tile.TileContext(nc) as tc:
        with tc.tile_pool(name="persist", bufs=1) as persist:
            qt_all = persist.tile([128, 2 * S], F16, tag="qt", name="qt")
            kt_all = persist.tile([128, 2 * S], F16, tag="kt", name="kt")
            vaug = [persist.tile([128, 2048], F16, tag=f"vaug{p}", name=f"vaug{p}")
                    for p in range(N_PAIRS)]
            # stack2[p]: [64tp+d, 512qh+128tt+r] = 16*O^T[d, q''] with
            # q'' = 1024qh + 128(2tt+tp) + r, tt in [0,4)
            stack2 = [persist.tile([128, 1024], F16, tag=f"stk{p}", name=f"stk{p}")
                      for p in range(N_PAIRS)]
            wo_sb = persist.tile([128, 8192], F16, tag="wo", name="wo")
            maskc_sb = [persist.tile([128, S], F16, tag=f"mask{t}", name=f"mask{t}")
                        for t in range(16)]


            # ---------------- Phase 1: projections ----------------
            with tc.tile_pool(name="asm_sb", bufs=1) as asmp:
                qsb = [asmp.tile([128, 1536], F16, tag=f"qsb{j}", name=f"qsb{j}")
                       for j in range(8)]
                ksb = [asmp.tile([128, 1536], F16, tag=f"ksb{j}", name=f"ksb{j}")
                       for j in range(8)]
                vsb = [asmp.tile([128, 1536], F16, tag=f"vsb{j}", name=f"vsb{j}")
                       for j in range(8)]
                def q2(j):
                    return nc.sync if j % 2 == 0 else nc.scalar
                for j in range(8):
                    q2(j).dma_start(out=qsb[j][:, :], in_=qasm[j])
                for j in range(8):
                    q2(j + 1).dma_start(out=ksb[j][:, :], in_=kasm[j])
                def mask_stage(t):
                    # mask t's u8 bytes stage in tile t+1's back half
                    # (t=15 in stack2[0]); conv(t+1)'s overwrite of that
                    # region serializes the chain in order
                    if t < 15:
                        return maskc_sb[t + 1][:, :].bitcast(
                            mybir.dt.uint8)[:, S:2 * S]
                    return stack2[0][:, :].bitcast(mybir.dt.uint8)

                def load_mask(t):
                    nc.gpsimd.dma_start(out=mask_stage(t), in_=maskc_d[t::16, :])

                for j in range(4):
                    nc.gpsimd.dma_start(out=vsb[j][:, :], in_=vasm[j])
                for t in range(3):
                    load_mask(t)
                for j in range(4, 8):
                    nc.gpsimd.dma_start(out=vsb[j][:, :], in_=vasm[j])
                for t in range(3, 16):
                    load_mask(t)
                nc.sync.dma_start(out=wo_sb[:, :], in_=wodr[:, :])
                for p in range(N_PAIRS):
                    va3 = vaug[p][:, :].rearrange("p (t c) -> p t c", c=128)
                    nc.gpsimd.memset(va3[:, :, 0:64], 1.0)
                def scatter_qk(ps, dst_all, tt, use_act):
                    # scatter-cast: psum[64sub+d, 256g+128hp+r]
                    #   -> dst[64hp+d, 2048g+128(2tt+sub)+r]
                    dst4 = dst_all[:, :].rearrange(
                        "p (g t r) -> p g t r", g=2, t=16)
                    for sub in range(2):
                        s4 = ps[64 * sub:64 * (sub + 1), :].rearrange(
                            "p (g h r) -> p g h r", g=2, h=2)
                        for hp in range(2):
                            d_ap = dst4[64 * hp:64 * (hp + 1), :, 2 * tt + sub, :]
                            s_ap = s4[:, :, hp, :]
                            if use_act:
                                nc.scalar.activation(
                                    d_ap, s_ap,
                                    mybir.ActivationFunctionType.Copy)
                            else:
                                nc.vector.tensor_copy(d_ap, s_ap)

                # Q: j-outer over 8 concurrent psum banks -- the first matmul
                # depends only on the first DMA chunk, so PE starts early
                with tc.tile_pool(name="qk_ps", bufs=8, space="PSUM") as qkps:
                    psq = [qkps.tile([128, 512], F32, tag="qk", name=f"psq{tt}")
                           for tt in range(8)]
                    for j in range(8):
                        for tt in range(8):
                            nc.tensor.matmul(
                                psq[tt][:, :],
                                lhsT=qsb[j][:, 128 * tt:128 * (tt + 1)],
                                rhs=qsb[j][:, 1024:1536],
                                start=(j == 0), stop=(j == 7))
                            if j == 7:
                                scatter_qk(psq[tt], qt_all, tt, use_act=False)
                    psk = [qkps.tile([128, 512], F32, tag="qk", name=f"psk{tt}")
                           for tt in range(8)]
                    for j in range(8):
                        for tt in range(8):
                            nc.tensor.matmul(
                                psk[tt][:, :],
                                lhsT=ksb[j][:, 128 * tt:128 * (tt + 1)],
                                rhs=ksb[j][:, 1024:1536],
                                start=(j == 0), stop=(j == 7))
                            if j == 7:
                                scatter_qk(psk[tt], kt_all, tt, use_act=True)

                    psv = [qkps.tile([128, 512], F32, tag="qk",
                                     name=f"psv{i}") for i in range(8)]
                    for j in range(8):
                        for p in range(N_PAIRS):
                            for oc in range(2):
                                nc.tensor.matmul(
                                    psv[2 * p + oc][:, :],
                                    lhsT=vsb[j][:, 1024 + 128 * p:1024 + 128 * (p + 1)],
                                    rhs=vsb[j][:, 512 * oc:512 * (oc + 1)],
                                    start=(j == 0), stop=(j == 7))
                    for p in range(N_PAIRS):
                        d3 = vaug[p][:, :].rearrange("p (t c) -> p t c", c=128)
                        for oc in range(2):
                            s3 = psv[2 * p + oc][:, :].rearrange(
                                "p (t c) -> p t c", c=64)
                            nc.vector.tensor_copy(
                                d3[:, 8 * oc:8 * (oc + 1), 64:128], s3)

            # ---------------- Phase 2: attention + output ----------------
            with tc.tile_pool(name="praw_p", bufs=3) as ppool, \
                 tc.tile_pool(name="pm_p", bufs=4) as pmpool, \
                 tc.tile_pool(name="norm", bufs=2) as npool, \
                 tc.tile_pool(name="outc", bufs=2) as opool, \
                 tc.tile_pool(name="st_ps", bufs=3, space="PSUM") as stps, \
                 tc.tile_pool(name="o_ps", bufs=1, space="PSUM") as ops:

                wo3 = wo_sb[:, :].rearrange("p (tt x) -> p tt x", tt=8)

                def emit_final(p, psF):
                    for qh in range(2):
                        for tt in range(4):
                            TT = 4 * qh + tt   # global t-pair = t//2
                            for oc in range(2):
                                nc.tensor.matmul(
                                    psF[:, 512 * oc:512 * (oc + 1)],
                                    lhsT=stack2[p][:, 512 * qh + 128 * tt:
                                                   512 * qh + 128 * (tt + 1)],
                                    rhs=wo3[:, TT, 512 * oc:512 * (oc + 1)],
                                    start=(qh == 0 and tt == 0),
                                    stop=(qh == 1 and tt == 3))
                    osb = opool.tile([128, 1024], F32, tag="osb", name="osb")
                    nc.vector.tensor_scalar_mul(osb[:, :], psF[:, :], 1.0 / 256.0)
                    oq = nc.sync if p % 2 == 0 else nc.scalar
                    oq.dma_start(out=out_d[128 * p:128 * (p + 1), :],
                                 in_=osb[:, :])

                pending_emit = [None]

                for p in range(N_PAIRS):
                    g, hp = p // 2, p % 2
                    lo, hi = 64 * hp, 64 * (hp + 1)
                    for qh in range(2):
                        psO = ops.tile([128, 1024], F32, tag="o", name="psO")
                        queue = []

                        def drain_one():
                            t, pm = queue.pop(0)
                            for sc in range(2):
                                nc.tensor.matmul(
                                    psO[:, 512 * sc:512 * (sc + 1)],
                                    lhsT=vaug[p][:, 128 * t:128 * (t + 1)],
                                    rhs=pm[:, 512 * sc:512 * (sc + 1)],
                                    start=(t == 0), stop=(t == 15))

                        for t in range(16):
                            stt = stps.tile([128, 1024], F32, tag="st", name="stt")
                            for sc in range(2):
                                nc.tensor.matmul(
                                    stt[:, 512 * sc:512 * (sc + 1)],
                                    lhsT=kt_all[lo:hi,
                                                2048 * g + 128 * t:2048 * g + 128 * (t + 1)],
                                    rhs=qt_all[lo:hi,
                                               2048 * g + 1024 * qh + 512 * sc:
                                               2048 * g + 1024 * qh + 512 * (sc + 1)],
                                    start=True, stop=True)
                            if p == 0 and qh == 0:
                                # pin behind the phase-1 casts: schedule only
                                # once the u8 DMA has actually landed
                                with tc.tile_wait_until(0.036 + 0.003 * t):
                                    nc.vector.tensor_scalar_mul(
                                        maskc_sb[t][:, :], mask_stage(t),
                                        MASK_SHIFT)
                            praw = ppool.tile([128, 1024], F16, tag="praw", name="praw")
                            nc.scalar.activation(praw[:, :], stt[:, :],
                                                 mybir.ActivationFunctionType.Exp,
                                                 scale=EXP_SCALE)
                            pm = pmpool.tile([128, 1024], F16, tag="pm", name="pm")
                            eng = nc.gpsimd if t in GP_STEPS else nc.vector
                            eng.tensor_mul(pm[:, :], praw[:, :],
                                           maskc_sb[t][:, 1024 * qh:1024 * (qh + 1)])
                            queue.append((t, pm))
                            if t == 2 and pending_emit[0] is not None:
                                emit_final(*pending_emit[0])
                                pending_emit[0] = None
                            if len(queue) > 2:
                                drain_one()
                        while queue:
                            drain_one()

                        # psO[0:64] = den copies, psO[64:128] = 16*O^T
                        recip = npool.tile([64, 1024], F32, tag="rc", name="recip")
                        nc.vector.reciprocal_approx_fast(recip[:, :], psO[0:64, :])
                        tmpn = npool.tile([128, 1024], F16, tag="tn", name="tmpn")
                        nc.vector.tensor_mul(tmpn[64:128, :], psO[64:128, :],
                                             recip[:, :])
                        # repack to stack2: even t -> partitions 0:64,
                        # odd t -> 64:128; cols compress 128tq'+r -> 128tt+r
                        src3 = tmpn[64:128, :].rearrange(
                            "p (tt tp r) -> p tt tp r", tt=4, tp=2)
                        for tp in range(2):
                            nc.vector.tensor_copy(
                                stack2[p][64 * tp:64 * (tp + 1),
                                          512 * qh:512 * (qh + 1)],
                                src3[:, :, tp, :])
                        if qh == 1:
                            pending_emit[0] = (p, ops.tile([128, 1024], F32,
                                                           tag="o", name="psF"))
                if pending_emit[0] is not None:
                    emit_final(*pending_emit[0])

    nc.finalize()
    return nc


def build_in_maps(inputs):
    q = np.asarray(inputs["q"], dtype=np.float32)
    k = np.asarray(inputs["k"], dtype=np.float32)
    v = np.asarray(inputs["v"], dtype=np.float32)
    mask = np.asarray(inputs["mask"])
    w_q = np.asarray(inputs["w_q"], dtype=np.float32)
    w_k = np.asarray(inputs["w_k"], dtype=np.float32)
    w_v = np.asarray(inputs["w_v"], dtype=np.float32)
    w_o = np.asarray(inputs["w_o"], dtype=np.float32)

    wqT = np.ascontiguousarray(w_q.T) * WSCALE
    wkT = np.ascontiguousarray(w_k.T) * WSCALE
    wvT = np.ascontiguousarray(w_v.T) * WSCALE
    wo16 = np.ascontiguousarray(w_o.T) * WSCALE      # [dm, c']
    # wodr[64tp+d, 1024tt + c'] = wo16[64(2tt+tp)+d, c']
    wodr = np.ascontiguousarray(
        wo16.reshape(8, 2, 64, D).transpose(1, 2, 0, 3).reshape(128, 8 * D)
    ).astype(np.float16)

    maskc = []
    for b in range(B):
        mt_ = (~mask[b]).T.astype(np.uint8)
        mp = mt_.reshape(S, 128, 16).transpose(0, 2, 1).reshape(S, S)
        maskc.append(np.ascontiguousarray(mp))

    in_maps = []
    for c in range(N_CORES):
        b, sb = c // 4, c % 4
        rows = slice(CORE_ROWS * sb, CORE_ROWS * (sb + 1))
        xqT = np.ascontiguousarray(q[b, rows].T)
        xkT = np.ascontiguousarray(k[b, rows].T)
        xvT = np.ascontiguousarray(v[b, rows].T)

        def pack(wT, xT):
            wc = wT.reshape(8, 128, D)
            xc = xT.reshape(8, 128, CORE_ROWS)
            return np.ascontiguousarray(
                np.concatenate([wc, xc], axis=2)).astype(np.float16)

        in_maps.append({
            "qasm": pack(wqT, xqT),
            "kasm": pack(wkT, xkT),
            "vasm": pack(wvT, xvT),
            "wodr": wodr,
            "maskc": maskc[b],
        })
    return in_maps


def kernel(q, k, v, mask, w_q, w_k, w_v, w_o):
    global _NC
    if _NC is None:
        _NC = _build_program()

    in_maps = build_in_maps(dict(q=q, k=k, v=v, mask=mask,
                                 w_q=w_q, w_k=w_k, w_v=w_v, w_o=w_o))
    res = run_bass_kernel_spmd(_NC, in_maps, list(range(N_CORES))).results

    out = np.empty((B, S, D), dtype=np.float32)
    for c in range(N_CORES):
        b, sb = c // 4, c % 4
        out[b, CORE_ROWS * sb:CORE_ROWS * (sb + 1)] = res[c]["out"]
    return out


# revision 31
# speedup vs baseline: 1.1784x; 1.1784x over previous
"""Multi-head attention (raw-reshape variant) on 8 trn2 NeuronCores.

Shapes: B=2, S=2048, D=1024, H=16, dh=64.  The reference uses a raw
reshape (B,S,D)->(B,H,S,dh), so head h only sees projected rows
[128h, 128h+128).  Core c handles b=c//4 and the 4 heads of seq-block
c%4.  No collectives.

All data tensors are fp16: softmax-weight quantization noise passes
through to the output at full strength (weights multiply V and both
signal and output shrink by the same averaging factor), so fp8 anywhere
on the value path costs ~3.6% output error -- over the 2e-2 gate.

v3 structure (vs the v1 baseline):
  1. Q/K proj with M=128 psum tiles [dm-pair-block, 512 rows] (half the
     PE passes of the M=64 parity-packed v1 layout); the fold to
     Qt/Kt[64hp+d, 2048g+128t+r] runs as 64 partition-moving
     psum->sbuf casts split between DVE and ACT.
  2. V proj natural [rows, dm] -> vaug fp16 with ones columns (the PV
     matmul emits the softmax denominator for free).
  3. Per (pair, q-half, t): St = Kt_t.T @ Qt (K=64); exp on ACT with
     the e^-4 softmax shift baked into the mask values; mask-mul on
     DVE (fp16 2x mode) with a few steps per phase on GPSIMD; PV
     accumulates psO (K=128).
  4. Normalize via DVE reciprocal + mul, then repack the stack to a
     two-t-block [128 = (tpar, d)] layout so the output projection
     contracts K=128 per pass; fused 1/256 rescale on the psum->sbuf
     copy; one wo load for the whole kernel.
"""

import numpy as np

import concourse.bass as bass
import concourse.mybir as mybir
import concourse.tile as tile
from concourse import bacc
from concourse.bass_utils import run_bass_kernel_spmd

F32 = mybir.dt.float32
F16 = mybir.dt.float16

B, S, D, H, DH = 2, 2048, 1024, 16, 64
N_CORES = 8
CORE_ROWS = 512
N_PAIRS = 4
WSCALE = 16.0
EXP_SCALE = 0.125 / (WSCALE * WSCALE)
MASK_SHIFT = float(np.exp(-4.0))
GP_STEPS = ()                    # gpsimd mask-mul offload (off: stalls PV)

_NC = None


def _build_program():
    nc = bacc.Bacc()

    # per contraction chunk k: [w chunk (1024 dm) | x chunk (512 rows)]
    qasm = nc.dram_tensor("qasm", [8, 128, 1536], F16, kind="ExternalInput")
    kasm = nc.dram_tensor("kasm", [8, 128, 1536], F16, kind="ExternalInput")
    vasm = nc.dram_tensor("vasm", [8, 128, 1536], F16, kind="ExternalInput")
    wodr = nc.dram_tensor("wodr", [128, 8192], F16, kind="ExternalInput")
    maskc_d = nc.dram_tensor("maskc", [S, S], mybir.dt.uint8, kind="ExternalInput")
    out_d = nc.dram_tensor("out", [CORE_ROWS, D], F32, kind="ExternalOutput")

    with tile.TileContext(nc) as tc:
        with tc.tile_pool(name="persist", bufs=1) as persist:
            qt_all = persist.tile([128, 2 * S], F16, tag="qt", name="qt")
            kt_all = persist.tile([128, 2 * S], F16, tag="kt", name="kt")
            vaug = [persist.tile([128, 2048], F16, tag=f"vaug{p}", name=f"vaug{p}")
                    for p in range(N_PAIRS)]
            # stack2[p]: [64tp+d, 512qh+128tt+r] = 16*O^T[d, q''] with
            # q'' = 1024qh + 128(2tt+tp) + r, tt in [0,4)
            stack2 = [persist.tile([128, 1024], F16, tag=f"stk{p}", name=f"stk{p}")
                      for p in range(N_PAIRS)]
            wo_sb = persist.tile([128, 8192], F16, tag="wo", name="wo")
            maskc_sb = [persist.tile([128, S], F16, tag=f"mask{t}", name=f"mask{t}")
                        for t in range(16)]


            # ---------------- Phase 1: projections ----------------
            with tc.tile_pool(name="asm_sb", bufs=1) as asmp:
                qsb = [asmp.tile([128, 1536], F16, tag=f"qsb{j}", name=f"qsb{j}")
                       for j in range(8)]
                ksb = [asmp.tile([128, 1536], F16, tag=f"ksb{j}", name=f"ksb{j}")
                       for j in range(8)]
                vsb = [asmp.tile([128, 1536], F16, tag=f"vsb{j}", name=f"vsb{j}")
                       for j in range(8)]
                def q2(j):
                    return nc.sync if j % 2 == 0 else nc.scalar
                for j in range(8):
                    q2(j).dma_start(out=qsb[j][:, :], in_=qasm[j])
                for j in range(8):
                    q2(j + 1).dma_start(out=ksb[j][:, :], in_=kasm[j])
                def mask_stage(t):
                    # mask t's u8 bytes stage in tile t+1's back half
                    # (t=15 in stack2[0]); conv(t+1)'s overwrite of that
                    # region serializes the chain in order
                    if t < 15:
                        return maskc_sb[t + 1][:, :].bitcast(
                            mybir.dt.uint8)[:, S:2 * S]
                    return stack2[0][:, :].bitcast(mybir.dt.uint8)

                def load_mask(t):
                    nc.gpsimd.dma_start(out=mask_stage(t), in_=maskc_d[t::16, :])

                for j in range(4):
                    nc.gpsimd.dma_start(out=vsb[j][:, :], in_=vasm[j])
                for t in range(3):
                    load_mask(t)
                for j in range(4, 8):
                    nc.gpsimd.dma_start(out=vsb[j][:, :], in_=vasm[j])
                for t in range(3, 16):
                    load_mask(t)
                nc.sync.dma_start(out=wo_sb[:, :], in_=wodr[:, :])
                for p in range(N_PAIRS):
                    va3 = vaug[p][:, :].rearrange("p (t c) -> p t c", c=128)
                    nc.gpsimd.memset(va3[:, :, 0:64], 1.0)
                def scatter_qk(ps, dst_all, tt, use_act):
                    # scatter-cast: psum[64sub+d, 256g+128hp+r]
                    #   -> dst[64hp+d, 2048g+128(2tt+sub)+r]
                    dst4 = dst_all[:, :].rearrange(
                        "p (g t r) -> p g t r", g=2, t=16)
                    for sub in range(2):
                        s4 = ps[64 * sub:64 * (sub + 1), :].rearrange(
                            "p (g h r) -> p g h r", g=2, h=2)
                        for hp in range(2):
                            d_ap = dst4[64 * hp:64 * (hp + 1), :, 2 * tt + sub, :]
                            s_ap = s4[:, :, hp, :]
                            if use_act:
                                nc.scalar.activation(
                                    d_ap, s_ap,
                                    mybir.ActivationFunctionType.Copy)
                            else:
                                nc.vector.tensor_copy(d_ap, s_ap)

                # Q: j-outer over 8 concurrent psum banks -- the first matmul
                # depends only on the first DMA chunk, so PE starts early
                with tc.tile_pool(name="qk_ps", bufs=8, space="PSUM") as qkps:
                    psq = [qkps.tile([128, 512], F32, tag="qk", name=f"psq{tt}")
                           for tt in range(8)]
                    for j in range(8):
                        for tt in range(8):
                            nc.tensor.matmul(
                                psq[tt][:, :],
                                lhsT=qsb[j][:, 128 * tt:128 * (tt + 1)],
                                rhs=qsb[j][:, 1024:1536],
                                start=(j == 0), stop=(j == 7))
                            if j == 7:
                                scatter_qk(psq[tt], qt_all, tt, use_act=False)
                    psk = [qkps.tile([128, 512], F32, tag="qk", name=f"psk{tt}")
                           for tt in range(8)]
                    for j in range(8):
                        for tt in range(8):
                            nc.tensor.matmul(
                                psk[tt][:, :],
                                lhsT=ksb[j][:, 128 * tt:128 * (tt + 1)],
                                rhs=ksb[j][:, 1024:1536],
                                start=(j == 0), stop=(j == 7))
                            if j == 7:
                                scatter_qk(psk[tt], kt_all, tt, use_act=True)

                    psv = [qkps.tile([128, 512], F32, tag="qk",
                                     name=f"psv{i}") for i in range(8)]
                    for j in range(8):
                        for p in range(N_PAIRS):
                            for oc in range(2):
                                nc.tensor.matmul(
                                    psv[2 * p + oc][:, :],
                                    lhsT=vsb[j][:, 1024 + 128 * p:1024 + 128 * (p + 1)],
                                    rhs=vsb[j][:, 512 * oc:512 * (oc + 1)],
                                    start=(j == 0), stop=(j == 7))
                    for p in range(N_PAIRS):
                        d3 = vaug[p][:, :].rearrange("p (t c) -> p t c", c=128)
                        for oc in range(2):
                            s3 = psv[2 * p + oc][:, :].rearrange(
                                "p (t c) -> p t c", c=64)
                            nc.vector.tensor_copy(
                                d3[:, 8 * oc:8 * (oc + 1), 64:128], s3)

            # ---------------- Phase 2: attention + output ----------------
            with tc.tile_pool(name="praw_p", bufs=3) as ppool, \
                 tc.tile_pool(name="pm_p", bufs=4) as pmpool, \
                 tc.tile_pool(name="norm", bufs=2) as npool, \
                 tc.tile_pool(name="outc", bufs=2) as opool, \
                 tc.tile_pool(name="st_ps", bufs=3, space="PSUM") as stps, \
                 tc.tile_pool(name="o_ps", bufs=1, space="PSUM") as ops:

                wo3 = wo_sb[:, :].rearrange("p (tt x) -> p tt x", tt=8)

                def emit_final(p, psF):
                    for qh in range(2):
                        for tt in range(4):
                            TT = 4 * qh + tt   # global t-pair = t//2
                            for oc in range(2):
                                nc.tensor.matmul(
                                    psF[:, 512 * oc:512 * (oc + 1)],
                                    lhsT=stack2[p][:, 512 * qh + 128 * tt:
                                                   512 * qh + 128 * (tt + 1)],
                                    rhs=wo3[:, TT, 512 * oc:512 * (oc + 1)],
                                    start=(qh == 0 and tt == 0),
                                    stop=(qh == 1 and tt == 3))
                    osb = opool.tile([128, 1024], F32, tag="osb", name="osb")
                    nc.vector.tensor_scalar_mul(osb[:, :], psF[:, :], 1.0 / 256.0)
                    nc.sync.dma_start(out=out_d[128 * p:128 * (p + 1), 0:512],
                                      in_=osb[:, 0:512])
                    nc.scalar.dma_start(out=out_d[128 * p:128 * (p + 1), 512:1024],
                                        in_=osb[:, 512:1024])

                pending_emit = [None]

                for p in range(N_PAIRS):
                    g, hp = p // 2, p % 2
                    lo, hi = 64 * hp, 64 * (hp + 1)
                    for qh in range(2):
                        psO = ops.tile([128, 1024], F32, tag="o", name="psO")
                        queue = []

                        def drain_one():
                            t, pm = queue.pop(0)
                            for sc in range(2):
                                nc.tensor.matmul(
                                    psO[:, 512 * sc:512 * (sc + 1)],
                                    lhsT=vaug[p][:, 128 * t:128 * (t + 1)],
                                    rhs=pm[:, 512 * sc:512 * (sc + 1)],
                                    start=(t == 0), stop=(t == 15))

                        for t in range(16):
                            stt = stps.tile([128, 1024], F32, tag="st", name="stt")
                            for sc in range(2):
                                nc.tensor.matmul(
                                    stt[:, 512 * sc:512 * (sc + 1)],
                                    lhsT=kt_all[lo:hi,
                                                2048 * g + 128 * t:2048 * g + 128 * (t + 1)],
                                    rhs=qt_all[lo:hi,
                                               2048 * g + 1024 * qh + 512 * sc:
                                               2048 * g + 1024 * qh + 512 * (sc + 1)],
                                    start=True, stop=True)
                            if p == 0 and qh == 0:
                                # pin behind the phase-1 casts: schedule only
                                # once the u8 DMA has actually landed
                                with tc.tile_wait_until(0.036 + 0.003 * t):
                                    nc.vector.tensor_scalar_mul(
                                        maskc_sb[t][:, :], mask_stage(t),
                                        MASK_SHIFT)
                            praw = ppool.tile([128, 1024], F16, tag="praw", name="praw")
                            nc.scalar.activation(praw[:, :], stt[:, :],
                                                 mybir.ActivationFunctionType.Exp,
                                                 scale=EXP_SCALE)
                            pm = pmpool.tile([128, 1024], F16, tag="pm", name="pm")
                            eng = nc.gpsimd if t in GP_STEPS else nc.vector
                            eng.tensor_mul(pm[:, :], praw[:, :],
                                           maskc_sb[t][:, 1024 * qh:1024 * (qh + 1)])
                            queue.append((t, pm))
                            if t == 2 and pending_emit[0] is not None:
                                emit_final(*pending_emit[0])
                                pending_emit[0] = None
                            if len(queue) > 2:
                                drain_one()
                        while queue:
                            drain_one()

                        # psO[0:64] = den copies, psO[64:128] = 16*O^T
                        recip = npool.tile([64, 1024], F32, tag="rc", name="recip")
                        nc.vector.reciprocal_approx_fast(recip[:, :], psO[0:64, :])
                        tmpn = npool.tile([128, 1024], F16, tag="tn", name="tmpn")
                        nc.vector.tensor_mul(tmpn[64:128, :], psO[64:128, :],
                                             recip[:, :])
                        # repack to stack2: even t -> partitions 0:64,
                        # odd t -> 64:128; cols compress 128tq'+r -> 128tt+r
                        src3 = tmpn[64:128, :].rearrange(
                            "p (tt tp r) -> p tt tp r", tt=4, tp=2)
                        for tp in range(2):
                            nc.vector.tensor_copy(
                                stack2[p][64 * tp:64 * (tp + 1),
                                          512 * qh:512 * (qh + 1)],
                                src3[:, :, tp, :])
                        if qh == 1:
                            pending_emit[0] = (p, stps.tile([128, 1024], F32,
                                                            tag="st", name="psF"))
                if pending_emit[0] is not None:
                    emit_final(*pending_emit[0])

    nc.finalize()
    return nc


def build_in_maps(inputs):
    q = np.asarray(inputs["q"], dtype=np.float32)
    k = np.asarray(inputs["k"], dtype=np.float32)
    v = np.asarray(inputs["v"], dtype=np.float32)
    mask = np.asarray(inputs["mask"])
    w_q = np.asarray(inputs["w_q"], dtype=np.float32)
    w_k = np.asarray(inputs["w_k"], dtype=np.float32)
    w_v = np.asarray(inputs["w_v"], dtype=np.float32)
    w_o = np.asarray(inputs["w_o"], dtype=np.float32)

    wqT = np.ascontiguousarray(w_q.T) * WSCALE
    wkT = np.ascontiguousarray(w_k.T) * WSCALE
    wvT = np.ascontiguousarray(w_v.T) * WSCALE
    wo16 = np.ascontiguousarray(w_o.T) * WSCALE      # [dm, c']
    # wodr[64tp+d, 1024tt + c'] = wo16[64(2tt+tp)+d, c']
    wodr = np.ascontiguousarray(
        wo16.reshape(8, 2, 64, D).transpose(1, 2, 0, 3).reshape(128, 8 * D)
    ).astype(np.float16)

    maskc = []
    for b in range(B):
        mt_ = (~mask[b]).T.astype(np.uint8)
        mp = mt_.reshape(S, 128, 16).transpose(0, 2, 1).reshape(S, S)
        maskc.append(np.ascontiguousarray(mp))

    in_maps = []
    for c in range(N_CORES):
        b, sb = c // 4, c % 4
        rows = slice(CORE_ROWS * sb, CORE_ROWS * (sb + 1))
        xqT = np.ascontiguousarray(q[b, rows].T)
        xkT = np.ascontiguousarray(k[b, rows].T)
        xvT = np.ascontiguousarray(v[b, rows].T)

        def pack(wT, xT):
            wc = wT.reshape(8, 128, D)
            xc = xT.reshape(8, 128, CORE_ROWS)
            return np.ascontiguousarray(
                np.concatenate([wc, xc], axis=2)).astype(np.float16)

        in_maps.append({
            "qasm": pack(wqT, xqT),
            "kasm": pack(wkT, xkT),
            "vasm": pack(wvT, xvT),
            "wodr": wodr,
            "maskc": maskc[b],
        })
    return in_maps


def kernel(q, k, v, mask, w_q, w_k, w_v, w_o):
    global _NC
    if _NC is None:
        _NC = _build_program()

    in_maps = build_in_maps(dict(q=q, k=k, v=v, mask=mask,
                                 w_q=w_q, w_k=w_k, w_v=w_v, w_o=w_o))
    res = run_bass_kernel_spmd(_NC, in_maps, list(range(N_CORES))).results

    out = np.empty((B, S, D), dtype=np.float32)
    for c in range(N_CORES):
        b, sb = c // 4, c % 4
        out[b, CORE_ROWS * sb:CORE_ROWS * (sb + 1)] = res[c]["out"]
    return out


# revision 32
# speedup vs baseline: 1.1901x; 1.0099x over previous
"""Multi-head attention (raw-reshape variant) on 8 trn2 NeuronCores.

Shapes: B=2, S=2048, D=1024, H=16, dh=64.  The reference uses a raw
reshape (B,S,D)->(B,H,S,dh), so head h only sees projected rows
[128h, 128h+128).  Core c handles b=c//4 and the 4 heads of seq-block
c%4.  No collectives.

All data tensors are fp16: softmax-weight quantization noise passes
through to the output at full strength (weights multiply V and both
signal and output shrink by the same averaging factor), so fp8 anywhere
on the value path costs ~3.6% output error -- over the 2e-2 gate.

v3 structure (vs the v1 baseline):
  1. Q/K proj with M=128 psum tiles [dm-pair-block, 512 rows] (half the
     PE passes of the M=64 parity-packed v1 layout); the fold to
     Qt/Kt[64hp+d, 2048g+128t+r] runs as 64 partition-moving
     psum->sbuf casts split between DVE and ACT.
  2. V proj natural [rows, dm] -> vaug fp16 with ones columns (the PV
     matmul emits the softmax denominator for free).
  3. Per (pair, q-half, t): St = Kt_t.T @ Qt (K=64); exp on ACT with
     the e^-4 softmax shift baked into the mask values; mask-mul on
     DVE (fp16 2x mode) with a few steps per phase on GPSIMD; PV
     accumulates psO (K=128).
  4. Normalize via DVE reciprocal + mul, then repack the stack to a
     two-t-block [128 = (tpar, d)] layout so the output projection
     contracts K=128 per pass; fused 1/256 rescale on the psum->sbuf
     copy; one wo load for the whole kernel.
"""

import numpy as np

import concourse.bass as bass
import concourse.mybir as mybir
import concourse.tile as tile
from concourse import bacc
from concourse.bass_utils import run_bass_kernel_spmd

F32 = mybir.dt.float32
F16 = mybir.dt.float16

B, S, D, H, DH = 2, 2048, 1024, 16, 64
N_CORES = 8
CORE_ROWS = 512
N_PAIRS = 4
WSCALE = 16.0
EXP_SCALE = 0.125 / (WSCALE * WSCALE)
MASK_SHIFT = float(np.exp(-4.0))
GP_STEPS = ()                    # gpsimd mask-mul offload (off: stalls PV)

_NC = None


def _build_program():
    nc = bacc.Bacc()

    # per contraction chunk k: [w chunk (1024 dm) | x chunk (512 rows)]
    qasm = nc.dram_tensor("qasm", [8, 128, 1536], F16, kind="ExternalInput")
    kasm = nc.dram_tensor("kasm", [8, 128, 1536], F16, kind="ExternalInput")
    vasm = nc.dram_tensor("vasm", [8, 128, 1536], F16, kind="ExternalInput")
    wodr = nc.dram_tensor("wodr", [128, 8192], F16, kind="ExternalInput")
    maskc_d = nc.dram_tensor("maskc", [S, S], mybir.dt.uint8, kind="ExternalInput")
    out_d = nc.dram_tensor("out", [CORE_ROWS, D], F32, kind="ExternalOutput")

    with tile.TileContext(nc) as tc:
        with tc.tile_pool(name="persist", bufs=1) as persist:
            qt_all = persist.tile([128, 2 * S], F16, tag="qt", name="qt")
            kt_all = persist.tile([128, 2 * S], F16, tag="kt", name="kt")
            vaug = [persist.tile([128, 2048], F16, tag=f"vaug{p}", name=f"vaug{p}")
                    for p in range(N_PAIRS)]
            # stack2[p]: [64tp+d, 512qh+128tt+r] = 16*O^T[d, q''] with
            # q'' = 1024qh + 128(2tt+tp) + r, tt in [0,4)
            stack2 = [persist.tile([128, 1024], F16, tag=f"stk{p}", name=f"stk{p}")
                      for p in range(N_PAIRS)]
            wo_sb = persist.tile([128, 8192], F16, tag="wo", name="wo")
            maskc_sb = [persist.tile([128, S], F16, tag=f"mask{t}", name=f"mask{t}")
                        for t in range(16)]


            # ---------------- Phase 1: projections ----------------
            with tc.tile_pool(name="asm_sb", bufs=1) as asmp:
                qsb = [asmp.tile([128, 1536], F16, tag=f"qsb{j}", name=f"qsb{j}")
                       for j in range(8)]
                ksb = [asmp.tile([128, 1536], F16, tag=f"ksb{j}", name=f"ksb{j}")
                       for j in range(8)]
                vsb = [asmp.tile([128, 1536], F16, tag=f"vsb{j}", name=f"vsb{j}")
                       for j in range(8)]
                def q2(j):
                    return nc.sync if j % 2 == 0 else nc.scalar
                for j in range(8):
                    q2(j).dma_start(out=qsb[j][:, :], in_=qasm[j])
                for j in range(8):
                    q2(j + 1).dma_start(out=ksb[j][:, :], in_=kasm[j])
                def mask_stage(t):
                    # mask t's u8 bytes stage in tile t+1's back half
                    # (t=15 in stack2[0]); conv(t+1)'s overwrite of that
                    # region serializes the chain in order
                    if t < 15:
                        return maskc_sb[t + 1][:, :].bitcast(
                            mybir.dt.uint8)[:, S:2 * S]
                    return stack2[0][:, :].bitcast(mybir.dt.uint8)

                def load_mask(t):
                    nc.gpsimd.dma_start(out=mask_stage(t), in_=maskc_d[t::16, :])

                for j in range(4):
                    nc.gpsimd.dma_start(out=vsb[j][:, :], in_=vasm[j])
                for t in range(3):
                    load_mask(t)
                for j in range(4, 8):
                    nc.gpsimd.dma_start(out=vsb[j][:, :], in_=vasm[j])
                for t in range(3, 16):
                    load_mask(t)
                nc.sync.dma_start(out=wo_sb[:, :], in_=wodr[:, :])
                for p in range(N_PAIRS):
                    va3 = vaug[p][:, :].rearrange("p (t c) -> p t c", c=128)
                    nc.gpsimd.memset(va3[:, :, 0:64], 1.0)
                def scatter_qk(ps, dst_all, tt, use_act):
                    # scatter-cast: psum[64sub+d, 256g+128hp+r]
                    #   -> dst[64hp+d, 2048g+128(2tt+sub)+r]
                    dst4 = dst_all[:, :].rearrange(
                        "p (g t r) -> p g t r", g=2, t=16)
                    for sub in range(2):
                        s4 = ps[64 * sub:64 * (sub + 1), :].rearrange(
                            "p (g h r) -> p g h r", g=2, h=2)
                        for hp in range(2):
                            d_ap = dst4[64 * hp:64 * (hp + 1), :, 2 * tt + sub, :]
                            s_ap = s4[:, :, hp, :]
                            if use_act:
                                nc.scalar.activation(
                                    d_ap, s_ap,
                                    mybir.ActivationFunctionType.Copy)
                            else:
                                nc.vector.tensor_copy(d_ap, s_ap)

                # Q: j-outer over 8 concurrent psum banks -- the first matmul
                # depends only on the first DMA chunk, so PE starts early
                with tc.tile_pool(name="qk_ps", bufs=8, space="PSUM") as qkps:
                    psq = [qkps.tile([128, 512], F32, tag="qk", name=f"psq{tt}")
                           for tt in range(8)]
                    for j in range(8):
                        for tt in range(8):
                            nc.tensor.matmul(
                                psq[tt][:, :],
                                lhsT=qsb[j][:, 128 * tt:128 * (tt + 1)],
                                rhs=qsb[j][:, 1024:1536],
                                start=(j == 0), stop=(j == 7))
                            if j == 7:
                                scatter_qk(psq[tt], qt_all, tt, use_act=False)
                    psk = [qkps.tile([128, 512], F32, tag="qk", name=f"psk{tt}")
                           for tt in range(8)]
                    for j in range(8):
                        for tt in range(8):
                            nc.tensor.matmul(
                                psk[tt][:, :],
                                lhsT=ksb[j][:, 128 * tt:128 * (tt + 1)],
                                rhs=ksb[j][:, 1024:1536],
                                start=(j == 0), stop=(j == 7))
                            if j == 7:
                                scatter_qk(psk[tt], kt_all, tt, use_act=True)

                    psv = [qkps.tile([128, 512], F32, tag="qk",
                                     name=f"psv{i}") for i in range(8)]
                    for j in range(8):
                        for p in range(N_PAIRS):
                            for oc in range(2):
                                nc.tensor.matmul(
                                    psv[2 * p + oc][:, :],
                                    lhsT=vsb[j][:, 1024 + 128 * p:1024 + 128 * (p + 1)],
                                    rhs=vsb[j][:, 512 * oc:512 * (oc + 1)],
                                    start=(j == 0), stop=(j == 7))
                    for p in range(N_PAIRS):
                        d3 = vaug[p][:, :].rearrange("p (t c) -> p t c", c=128)
                        for oc in range(2):
                            s3 = psv[2 * p + oc][:, :].rearrange(
                                "p (t c) -> p t c", c=64)
                            nc.vector.tensor_copy(
                                d3[:, 8 * oc:8 * (oc + 1), 64:128], s3)

            # ---------------- Phase 2: attention + output ----------------
            with tc.tile_pool(name="praw_p", bufs=3) as ppool, \
                 tc.tile_pool(name="pm_p", bufs=6) as pmpool, \
                 tc.tile_pool(name="norm", bufs=2) as npool, \
                 tc.tile_pool(name="outc", bufs=2) as opool, \
                 tc.tile_pool(name="st_ps", bufs=3, space="PSUM") as stps, \
                 tc.tile_pool(name="o_ps", bufs=1, space="PSUM") as ops:

                wo3 = wo_sb[:, :].rearrange("p (tt x) -> p tt x", tt=8)

                def emit_final(p, psF):
                    for qh in range(2):
                        for tt in range(4):
                            TT = 4 * qh + tt   # global t-pair = t//2
                            for oc in range(2):
                                nc.tensor.matmul(
                                    psF[:, 512 * oc:512 * (oc + 1)],
                                    lhsT=stack2[p][:, 512 * qh + 128 * tt:
                                                   512 * qh + 128 * (tt + 1)],
                                    rhs=wo3[:, TT, 512 * oc:512 * (oc + 1)],
                                    start=(qh == 0 and tt == 0),
                                    stop=(qh == 1 and tt == 3))
                    osb = opool.tile([128, 1024], F32, tag="osb", name="osb")
                    nc.vector.tensor_scalar_mul(osb[:, :], psF[:, :], 1.0 / 256.0)
                    nc.sync.dma_start(out=out_d[128 * p:128 * (p + 1), 0:512],
                                      in_=osb[:, 0:512])
                    nc.scalar.dma_start(out=out_d[128 * p:128 * (p + 1), 512:1024],
                                        in_=osb[:, 512:1024])

                pending_emit = [None]

                for p in range(N_PAIRS):
                    g, hp = p // 2, p % 2
                    lo, hi = 64 * hp, 64 * (hp + 1)
                    for qh in range(2):
                        psO = ops.tile([128, 1024], F32, tag="o", name="psO")
                        queue = []

                        def drain_one():
                            t, pm = queue.pop(0)
                            for sc in range(2):
                                nc.tensor.matmul(
                                    psO[:, 512 * sc:512 * (sc + 1)],
                                    lhsT=vaug[p][:, 128 * t:128 * (t + 1)],
                                    rhs=pm[:, 512 * sc:512 * (sc + 1)],
                                    start=(t == 0), stop=(t == 15))

                        for t in range(16):
                            stt = stps.tile([128, 1024], F32, tag="st", name="stt")
                            for sc in range(2):
                                nc.tensor.matmul(
                                    stt[:, 512 * sc:512 * (sc + 1)],
                                    lhsT=kt_all[lo:hi,
                                                2048 * g + 128 * t:2048 * g + 128 * (t + 1)],
                                    rhs=qt_all[lo:hi,
                                               2048 * g + 1024 * qh + 512 * sc:
                                               2048 * g + 1024 * qh + 512 * (sc + 1)],
                                    start=True, stop=True)
                            if p == 0 and qh == 0:
                                # pin behind the phase-1 casts: schedule only
                                # once the u8 DMA has actually landed
                                with tc.tile_wait_until(0.036 + 0.003 * t):
                                    nc.vector.tensor_scalar_mul(
                                        maskc_sb[t][:, :], mask_stage(t),
                                        MASK_SHIFT)
                            praw = ppool.tile([128, 1024], F16, tag="praw", name="praw")
                            nc.scalar.activation(praw[:, :], stt[:, :],
                                                 mybir.ActivationFunctionType.Exp,
                                                 scale=EXP_SCALE)
                            pm = pmpool.tile([128, 1024], F16, tag="pm", name="pm")
                            eng = nc.gpsimd if t in GP_STEPS else nc.vector
                            eng.tensor_mul(pm[:, :], praw[:, :],
                                           maskc_sb[t][:, 1024 * qh:1024 * (qh + 1)])
                            queue.append((t, pm))
                            if t == 2 and pending_emit[0] is not None:
                                emit_final(*pending_emit[0])
                                pending_emit[0] = None
                            if len(queue) > 4:
                                drain_one()
                        while queue:
                            drain_one()

                        # psO[0:64] = den copies, psO[64:128] = 16*O^T
                        recip = npool.tile([64, 1024], F32, tag="rc", name="recip")
                        nc.vector.reciprocal_approx_fast(recip[:, :], psO[0:64, :])
                        tmpn = npool.tile([128, 1024], F16, tag="tn", name="tmpn")
                        nc.vector.tensor_mul(tmpn[64:128, :], psO[64:128, :],
                                             recip[:, :])
                        # repack to stack2: even t -> partitions 0:64,
                        # odd t -> 64:128; cols compress 128tq'+r -> 128tt+r
                        src3 = tmpn[64:128, :].rearrange(
                            "p (tt tp r) -> p tt tp r", tt=4, tp=2)
                        for tp in range(2):
                            nc.vector.tensor_copy(
                                stack2[p][64 * tp:64 * (tp + 1),
                                          512 * qh:512 * (qh + 1)],
                                src3[:, :, tp, :])
                        if qh == 1:
                            pending_emit[0] = (p, stps.tile([128, 1024], F32,
                                                            tag="st", name="psF"))
                if pending_emit[0] is not None:
                    emit_final(*pending_emit[0])

    nc.finalize()
    return nc


def build_in_maps(inputs):
    q = np.asarray(inputs["q"], dtype=np.float32)
    k = np.asarray(inputs["k"], dtype=np.float32)
    v = np.asarray(inputs["v"], dtype=np.float32)
    mask = np.asarray(inputs["mask"])
    w_q = np.asarray(inputs["w_q"], dtype=np.float32)
    w_k = np.asarray(inputs["w_k"], dtype=np.float32)
    w_v = np.asarray(inputs["w_v"], dtype=np.float32)
    w_o = np.asarray(inputs["w_o"], dtype=np.float32)

    wqT = np.ascontiguousarray(w_q.T) * WSCALE
    wkT = np.ascontiguousarray(w_k.T) * WSCALE
    wvT = np.ascontiguousarray(w_v.T) * WSCALE
    wo16 = np.ascontiguousarray(w_o.T) * WSCALE      # [dm, c']
    # wodr[64tp+d, 1024tt + c'] = wo16[64(2tt+tp)+d, c']
    wodr = np.ascontiguousarray(
        wo16.reshape(8, 2, 64, D).transpose(1, 2, 0, 3).reshape(128, 8 * D)
    ).astype(np.float16)

    maskc = []
    for b in range(B):
        mt_ = (~mask[b]).T.astype(np.uint8)
        mp = mt_.reshape(S, 128, 16).transpose(0, 2, 1).reshape(S, S)
        maskc.append(np.ascontiguousarray(mp))

    in_maps = []
    for c in range(N_CORES):
        b, sb = c // 4, c % 4
        rows = slice(CORE_ROWS * sb, CORE_ROWS * (sb + 1))
        xqT = np.ascontiguousarray(q[b, rows].T)
        xkT = np.ascontiguousarray(k[b, rows].T)
        xvT = np.ascontiguousarray(v[b, rows].T)

        def pack(wT, xT):
            wc = wT.reshape(8, 128, D)
            xc = xT.reshape(8, 128, CORE_ROWS)
            return np.ascontiguousarray(
                np.concatenate([wc, xc], axis=2)).astype(np.float16)

        in_maps.append({
            "qasm": pack(wqT, xqT),
            "kasm": pack(wkT, xkT),
            "vasm": pack(wvT, xvT),
            "wodr": wodr,
            "maskc": maskc[b],
        })
    return in_maps


def kernel(q, k, v, mask, w_q, w_k, w_v, w_o):
    global _NC
    if _NC is None:
        _NC = _build_program()

    in_maps = build_in_maps(dict(q=q, k=k, v=v, mask=mask,
                                 w_q=w_q, w_k=w_k, w_v=w_v, w_o=w_o))
    res = run_bass_kernel_spmd(_NC, in_maps, list(range(N_CORES))).results

    out = np.empty((B, S, D), dtype=np.float32)
    for c in range(N_CORES):
        b, sb = c // 4, c % 4
        out[b, CORE_ROWS * sb:CORE_ROWS * (sb + 1)] = res[c]["out"]
    return out
